# revision 16
# baseline (speedup 1.0000x reference)
"""Trainium2 Bass kernel for nn_Attention_82867099009253 (sparse_attention).

Tensor-parallel over heads (H=8 == 8 NeuronCores); each core computes one
head for all 4 batches:
  kv_in = depthwise_conv3(x^T) (chunked @1000, zero-pad) + x^T   [DVE engine]
  q = (Wq_h @ x^T) * hd^-0.5        (scale folded into host-side weights)
  k|v = [Wk_h; Wv_h] @ kv_in        (single-tap fused projection)
  S^T[k,m] = k^T q + rpe^T          (per 128-key chunk, psum f32; the rpe
                                     bias rides the same psum accumulation as
                                     an fp8e4m3 DoubleRow identity-matmul:
                                     tile0 = I*rpe_chunk, tile1 = 0*next)
  P^T = exp(S^T - 4)                (fp16, ACT engine; softmax max-
                                     subtraction skipped: |S|<~8)
  out[m,d] += P_chunk^T^T... PV computed TRANSPOSED: stationary = P^T
  chunk [keys, m-128], moving = v_aug [keys, 65] (v^T columns + ones col
  for the denominator) -> psum [m, 65]. v^T built by DMA-XBAR transposes.
  Host divides num/den and reassembles; the reference's flat reshape makes
  each head's [hd, L] block contiguous in the output.
All matmuls fp16 except the rpe-add (fp8, exact for the identity path and
3.6% relative on |rpe|<=0.1 values -> ~1e-3 effect on S).
"""

import os
import numpy as np
import ml_dtypes

import concourse.bass as bass
import concourse.bacc as bacc
import concourse.tile as tile
import concourse.mybir as mybir
from concourse.bass_utils import run_bass_kernel_spmd

F32 = mybir.dt.float32
F16 = mybir.dt.float16
F8 = mybir.dt.float8e4
Alu = mybir.AluOpType
Act = mybir.ActivationFunctionType
DR = mybir.MatmulPerfMode.DoubleRow
E4 = ml_dtypes.float8_e4m3

B, L, C, H = 4, 2000, 512, 8
HD = C // H            # 64
CH = 1000              # conv chunk
PW = 2 * CH + 4        # padded x width: [0 | ch0 | 0 0 | ch1 | 0]
NCH = 16               # 128-row key chunks (15*128 + 80)
EXPB = -4.0            # exp bias (p = exp(S + rpe + EXPB); cancels in ratio)
# m-halves: (m offset, width, S-matmul piece widths, PVT m-chunk widths)
MH = [(0, 1024, [(0, 512), (512, 512)], [128] * 8),
      (1024, 976, [(0, 512), (512, 464)], [128] * 7 + [80])]

LAST_EXEC_NS = None
LAST_RESULTS = None


def _cw(n):
    return 128 if n < NCH - 1 else L - 128 * (NCH - 1)


def _center_col(off):
    ch = off // CH
    return 1 + ch * (CH + 2) + (off - ch * CH)


def build_kernel(debug=False, repeat=1):
    nc = bacc.Bacc("TRN2")

    xpad_d = nc.dram_tensor("xpad", [B, C, PW], F16, kind="ExternalInput")
    rpe8_d = nc.dram_tensor("rpe8", [NCH, 128, L], F8, kind="ExternalInput")
    iz_d = nc.dram_tensor("iz", [128, 2, 128], F8, kind="ExternalInput")
    zi_d = nc.dram_tensor("zi", [128, 2, 128], F8, kind="ExternalInput")
    wq_d = nc.dram_tensor("wqT", [C, HD], F16, kind="ExternalInput")
    wkv_d = nc.dram_tensor("wkvT", [C, 128], F16, kind="ExternalInput")
    cwm_d = nc.dram_tensor("convw", [128, 12], F32, kind="ExternalInput")
    bq_d = nc.dram_tensor("biasq", [HD, 1], F32, kind="ExternalInput")
    bkv_d = nc.dram_tensor("biaskv", [128, 1], F32, kind="ExternalInput")
    out_d = nc.dram_tensor("outT", [B, 128, NCH, 65], F32, kind="ExternalOutput")
    if debug:
        kk_dbg = nc.dram_tensor("kk_dbg", [128, L], F16, kind="ExternalOutput")
        qq_dbg = nc.dram_tensor("qq_dbg", [128, L], F16, kind="ExternalOutput")
        vt_dbg = nc.dram_tensor("vt_dbg", [128, 2048], F16, kind="ExternalOutput")
        vb_dbg = nc.dram_tensor("vb_dbg", [128, NCH, 65], F16,
                                kind="ExternalOutput")
        pt_dbg = nc.dram_tensor("pt_dbg", [2, 128, 1024], F16,
                                kind="ExternalOutput")

    with tile.TileContext(nc) as tc:
        with (
            tc.tile_pool(name="const", bufs=1) as const,
            tc.tile_pool(name="xp", bufs=5) as xp_pool,
            tc.tile_pool(name="cvp", bufs=6) as cv_pool,
            tc.tile_pool(name="act2k", bufs=2) as act2k,
            tc.tile_pool(name="vb", bufs=2) as vb_pool,
            tc.tile_pool(name="pt", bufs=6) as pt_pool,
            tc.tile_pool(name="ob", bufs=2) as ob_pool,
            tc.tile_pool(name="ppp", bufs=1, space="PSUM") as pp,    # 2 banks
            tc.tile_pool(name="stp", bufs=2, space="PSUM") as stp,   # 4 banks
            tc.tile_pool(name="pvp", bufs=1, space="PSUM") as pvp,   # 2 banks
        ):
            # ---- persistent constants ----
            wq_sb = const.tile([128, 4, HD], F16)
            nc.sync.dma_start(wq_sb[:], wq_d[:].rearrange("(o p) d -> p o d", p=128))
            wkv_sb = const.tile([128, 4, 128], F16)
            nc.sync.dma_start(
                wkv_sb[:], wkv_d[:].rearrange("(o p) d -> p o d", p=128))
            cw_sb = const.tile([128, 12], F32)
            nc.sync.dma_start(cw_sb[:], cwm_d[:])
            bq_sb = const.tile([HD, 1], F32)
            nc.sync.dma_start(bq_sb[:], bq_d[:])
            bkv_sb = const.tile([128, 1], F32)
            nc.sync.dma_start(bkv_sb[:], bkv_d[:])
            nbias = const.tile([128, 1], F32)
            nc.vector.memset(nbias[:], EXPB)
            iz_sb = const.tile([128, 2, 128], F8)
            nc.sync.dma_start(iz_sb[:], iz_d[:])
            zi_sb = const.tile([128, 2, 128], F8)
            nc.sync.dma_start(zi_sb[:], zi_d[:])
            # rpe8: [128, NCH, L] fp8, split DMAs so chunk 0 lands early
            rpe8 = const.tile([128, NCH, L], F8)
            for n in range(NCH):
                for hh in range(2):
                    nc.sync.dma_start(
                        rpe8[:, n, 1000 * hh:1000 * hh + 1000],
                        rpe8_d[n, :, 1000 * hh:1000 * hh + 1000])

            def emit_conv_proj(b, _ctr=[0]):
                """Conv (DVE) + projections (PE). DMAs issued immediately;
                returns (tiles, steps) to interleave into attention chunks."""
                _ctr[0] += 1
                u = _ctr[0]
                xts = []
                for c in range(4):
                    xt = xp_pool.tile([128, PW], F16, tag="xp", name=f"xt{u}_{c}")
                    for piece in range(4):
                        nc.sync.dma_start(
                            xt[:, 501 * piece:501 * piece + 501],
                            xpad_d[b, 128 * c:128 * c + 128,
                                   501 * piece:501 * piece + 501])
                    xts.append(xt)

                kk = act2k.tile([128, L], F16, tag="kk", name=f"kk{u}")
                vt = act2k.tile([128, 2048], F16, tag="vt", name=f"vt{u}")
                qq = act2k.tile([128, L], F16, tag="qq", name=f"qq{u}")
                v_big = vb_pool.tile([128, NCH, 65], F16, tag="vb", name=f"vb{u}")
                kv_in = []
                steps = []

                def conv_chunk(c):
                    # kv_in[c][:, s, l] = conv3(x)[l of span s] (+x residual,
                    # folded into w2'); spans are the two zero-padded halves
                    def run(c=c):
                        t = cv_pool.tile([128, 2, CH], F16, tag="cv",
                                         name=f"cv{u}_{c}")
                        s = cv_pool.tile([128, 2, CH], F16, tag="cvs",
                                         name=f"cvs{u}_{c}")
                        # span s of xpad occupies cols [1002s, 1002s+1002);
                        # conv position l of span s, tap d reads col 1002s+l+d
                        xtv = xts[c][:].rearrange("p (s w) -> p s w", w=CH + 2)
                        xv = [xtv[:, :, d:d + CH] for d in range(3)]
                        w1 = cw_sb[:, 3 * c:3 * c + 1]
                        w2 = cw_sb[:, 3 * c + 1:3 * c + 2]
                        w3 = cw_sb[:, 3 * c + 2:3 * c + 3]
                        nc.vector.tensor_scalar(t[:], xv[1], w2, None, Alu.mult)
                        nc.vector.tensor_scalar(s[:], xv[0], w1, None, Alu.mult)
                        nc.vector.tensor_tensor(out=t[:], in0=t[:], in1=s[:],
                                                op=Alu.add)
                        nc.vector.tensor_scalar(s[:], xv[2], w3, None, Alu.mult)
                        nc.vector.tensor_tensor(out=t[:], in0=t[:], in1=s[:],
                                                op=Alu.add)
                        kv_in.append(t)
                    return run

                def kv_half(h):
                    ps_kv = pp.tile([128, 1024], F32, tag="pp", name=f"pskv{u}_{h}")
                    def mms(li, ps_kv=ps_kv, h=h):
                        lo = 500 * li + 1000 * h
                        col = 512 * li
                        for c in range(4):
                            nc.tensor.matmul(
                                ps_kv[:, col:col + 500],
                                wkv_sb[:, c, :],
                                kv_in[c][:, h, lo - 1000 * h:lo - 1000 * h + 500],
                                start=(c == 0), stop=(c == 3),
                            )
                    def copies(ps_kv=ps_kv, h=h):
                        pv_v = ps_kv[:].rearrange("p (l m) -> p l m", m=512)
                        kk_v = kk[:, 1000 * h:1000 * h + 1000].rearrange(
                            "p (l m) -> p l m", m=500)
                        vt_v = vt[:, 1000 * h:1000 * h + 1000].rearrange(
                            "p (l m) -> p l m", m=500)
                        nc.vector.tensor_scalar(
                            kk_v[0:HD], pv_v[0:HD, :, 0:500], bkv_sb[0:HD],
                            None, Alu.add)
                        nc.vector.tensor_scalar(
                            vt_v[HD:128], pv_v[HD:128, :, 0:500], bkv_sb[HD:128],
                            None, Alu.add)
                    return [lambda: mms(0), lambda: mms(1), copies]

                def q_half(h):
                    ps_q = pp.tile([128, 1024], F32, tag="pp", name=f"psq{u}_{h}")
                    def mms(ps_q=ps_q, h=h):
                        for li in range(2):
                            lo = 500 * li + 1000 * h
                            col = 512 * li
                            cc = _center_col(lo)
                            for c in range(4):
                                nc.tensor.matmul(
                                    ps_q[0:HD, col:col + 500],
                                    wq_sb[:, c, :],
                                    xts[c][:, cc:cc + 500],
                                    start=(c == 0), stop=(c == 3),
                                )
                    def copies(ps_q=ps_q, h=h):
                        pq_v = ps_q[:].rearrange("p (l m) -> p l m", m=512)
                        qq_v = qq[:, 1000 * h:1000 * h + 1000].rearrange(
                            "p (l m) -> p l m", m=500)
                        nc.vector.tensor_scalar(
                            qq_v[0:HD], pq_v[0:HD, :, 0:500], bq_sb[:], None,
                            Alu.add)
                    return [mms, copies]

                def vtr(v_big=v_big):
                    # XBAR transpose needs 4B-aligned dest offsets: land in a
                    # packed [128,16,64] tile, then strided-copy into the
                    # 65-wide v_aug layout next to the ones column
                    vb64 = cv_pool.tile([128, NCH, 64], F16, tag="vb64",
                                        name=f"vb64{u}")
                    nc.gpsimd.memset(v_big[:, :, 64:65], 1.0)
                    nc.gpsimd.memset(vt[HD:128, 2000:2048], 0.0)
                    for n in range(NCH):
                        nc.sync.dma_start_transpose(
                            vb64[:, n, :], vt[HD:128, 128 * n:128 * n + 128])
                    nc.vector.tensor_scalar(
                        v_big[:, :, 0:64], vb64[:], 0.0, None, Alu.add)

                for c in range(4):
                    steps.append(conv_chunk(c))
                steps += kv_half(0)
                steps += kv_half(1)
                steps.append(lambda: nc.sync.dma_start(kk[HD:128, :], kk[0:HD, :]))
                steps.append(vtr)
                steps += q_half(0)
                steps += q_half(1)
                steps.append(lambda: nc.sync.dma_start(qq[HD:128, :], qq[0:HD, :]))
                return (kk, qq, v_big), steps

            def emit_attention_half(b, mh_i, kk, qq, v_big, ot, _ctr=[0]):
                mo0, mw0, mms, mcw = MH[mh_i]
                _ctr[0] += 1
                u = _ctr[0]
                ps_out = pvp.tile([128, 2, 512], F32, tag="pv", name=f"po{u}")
                # explicit zero-init: matmul start=True zeroes the whole 2KB
                # psum bank on TRN2, and the dataflow scheduler may reorder
                # disjoint-region accumulating matmuls before it; a memset
                # gives every PVT matmul a tracked RAW dependency instead
                nc.vector.memset(ps_out[:], 0.0)
                for n in range(NCH):
                    w = _cw(n)
                    st = stp.tile([128, 1024], F32, tag="st", name=f"st{u}_{n}")
                    hp = HD * (n % 2)
                    for mo, mw in mms:
                        nc.tensor.matmul(
                            st[0:w, mo:mo + mw],
                            kk[hp:hp + HD, 128 * n:128 * n + w],
                            qq[hp:hp + HD, mo0 + mo:mo0 + mo + mw],
                            start=True, stop=False,
                        )
                    # rpe bias add: fp8 DoubleRow identity matmul, k-tile pair
                    # (n, n+1) with second tile zero-weighted (IZ), or
                    # (n-1, n) via ZI for the last chunk
                    lhs8 = iz_sb if n < NCH - 1 else zi_sb
                    n0 = n if n < NCH - 1 else n - 1
                    for mo, mw in mms:
                        nc.tensor.matmul(
                            st[0:w, mo:mo + mw],
                            lhs8[:, :, 0:w],
                            rpe8[:, n0:n0 + 2, mo0 + mo:mo0 + mo + mw],
                            start=False, stop=True, perf_mode=DR,
                            skip_group_check=True,
                        )
                    pt = pt_pool.tile([128, 1024], F16, tag="pt", name=f"pt{u}_{n}")
                    nc.scalar.activation(
                        pt[0:w, 0:mw0], st[0:w, 0:mw0], Act.Exp,
                        bias=nbias[0:w])
                    if debug and b == 0 and mh_i == 0 and n < 2:
                        nc.sync.dma_start(pt_dbg[n], pt[:])
                    # transposed PV: stationary = pt m-chunk, moving = v_aug
                    for j, mp in enumerate(mcw):
                        q4, j4 = divmod(j, 4)
                        nc.tensor.matmul(
                            ps_out[0:mp, q4, 65 * j4:65 * j4 + 65],
                            pt[0:w, 128 * j:128 * j + mp],
                            v_big[0:w, n, :],
                            start=False, stop=(n == NCH - 1),
                            skip_group_check=True,
                        )
                for q4 in range(2):
                    nc.vector.tensor_scalar(
                        ot[:, 8 * mh_i + 4 * q4:8 * mh_i + 4 * q4 + 4, :],
                        ps_out[:, q4, 0:260].rearrange("p (c w) -> p c w", w=65),
                        0.0, None, Alu.add)

            state, steps0 = emit_conv_proj(0)
            for st_fn in steps0:
                st_fn()
            if debug:
                nc.sync.dma_start(kk_dbg[:], state[0][:])
                nc.sync.dma_start(qq_dbg[:], state[1][:])
                nc.sync.dma_start(vb_dbg[:], state[2][:])
            for rep in range(repeat):
                for b in range(B):
                    kk, qq, v_big = state
                    ot = ob_pool.tile([128, NCH, 65], F32, tag="ot",
                                      name=f"ot{b}_{rep}")
                    emit_attention_half(b, 0, kk, qq, v_big, ot)
                    if b + 1 < B or rep + 1 < repeat:
                        state, steps = emit_conv_proj((b + 1) % B)
                        for st_fn in steps:
                            st_fn()
                    emit_attention_half(b, 1, kk, qq, v_big, ot)
                    nc.sync.dma_start(out_d[b], ot[:])

    nc.finalize()
    return nc


_NC_CACHE = None


def _get_nc():
    global _NC_CACHE
    if _NC_CACHE is None:
        _NC_CACHE = build_kernel()
    return _NC_CACHE


def _host_prep(x, rpe, Wq, bq, Wkv, bkv, Wl, bl):
    scale = float(HD) ** -0.5
    xt = np.ascontiguousarray(np.swapaxes(x, 1, 2))          # [B, C, L]
    xpad = np.zeros((B, C, PW), np.float16)
    xpad[:, :, 1:1 + CH] = xt[:, :, 0:CH]
    xpad[:, :, CH + 3:CH + 3 + CH] = xt[:, :, CH:L]

    w1 = Wl[:, 0, 0].astype(np.float64)
    w2 = Wl[:, 0, 1].astype(np.float64) + 1.0
    w3 = Wl[:, 0, 2].astype(np.float64)
    convw = np.zeros((128, 12), np.float32)
    for c in range(4):
        sl = slice(128 * c, 128 * c + 128)
        convw[:, 3 * c + 0] = w1[sl]
        convw[:, 3 * c + 1] = w2[sl]
        convw[:, 3 * c + 2] = w3[sl]

    bias_kv_full = (Wkv.astype(np.float64) @ bl.astype(np.float64)
                    + bkv.astype(np.float64))

    iz = np.zeros((128, 2, 128), E4)
    zi = np.zeros((128, 2, 128), E4)
    iz[:, 0][np.arange(128), np.arange(128)] = 1.0
    zi[:, 1][np.arange(128), np.arange(128)] = 1.0

    in_maps = []
    for h in range(H):
        r = slice(HD * h, HD * h + HD)
        rv = slice(C + HD * h, C + HD * h + HD)
        wqT = np.ascontiguousarray((Wq[r, :] * scale).T).astype(np.float16)
        wkvT = np.ascontiguousarray(
            np.concatenate([Wkv[r, :], Wkv[rv, :]], 0).T).astype(np.float16)
        biasq = (bq[r] * scale).astype(np.float32).reshape(HD, 1)
        biaskv = np.concatenate(
            [bias_kv_full[r], bias_kv_full[rv]]).astype(np.float32).reshape(128, 1)
        rpeT = np.zeros((NCH * 128, L), np.float32)
        rpeT[0:L] = rpe[0, h].T
        rpe8 = rpeT.reshape(NCH, 128, L).astype(E4)
        in_maps.append({
            "xpad": xpad, "rpe8": rpe8, "iz": iz, "zi": zi,
            "wqT": wqT, "wkvT": wkvT, "convw": convw,
            "biasq": biasq, "biaskv": biaskv,
        })
    return in_maps


def kernel(x, relative_pos_enc, Wq, bq, Wkv, bkv, Wl, bl):
    global LAST_EXEC_NS, LAST_RESULTS
    in_maps = _host_prep(np.asarray(x, np.float32),
                         np.asarray(relative_pos_enc, np.float32),
                         np.asarray(Wq, np.float32), np.asarray(bq, np.float32),
                         np.asarray(Wkv, np.float32), np.asarray(bkv, np.float32),
                         np.asarray(Wl, np.float32), np.asarray(bl, np.float32))
    nc = _get_nc()
    trace = bool(int(os.environ.get("KERNEL_TRACE", "0")))
    res = run_bass_kernel_spmd(nc, in_maps, core_ids=list(range(H)), trace=trace)
    LAST_EXEC_NS = res.exec_time_ns
    LAST_RESULTS = res
    arr = np.stack([res.results[h]["outT"] for h in range(H)], 0)
    # [H, B, 128, NCH, 65]: m = 128*chunk + p -> [H, B, L, 65]
    arr = arr.transpose(0, 1, 3, 2, 4).reshape(H, B, NCH * 128, 65)[:, :, 0:L]
    out_md = arr[:, :, :, 0:64] / arr[:, :, :, 64:65]
    out_t = np.ascontiguousarray(out_md.transpose(0, 1, 3, 2))  # [H, B, 64, L]
    out = np.ascontiguousarray(out_t.transpose(1, 0, 2, 3)).reshape(B, L, C)
    return out.astype(np.float32)
